# revision 20
# baseline (speedup 1.0000x reference)
"""Trainium2 Bass kernel for nn_CriticOld (twin-Q GNN critic: per-sample kNN +
EdgeConv + MLP head), data-parallel over batch across 8 NeuronCores.

Layout (per core): 128 "problems" = 2 Q-networks x 64 samples; col = prob*30 +
node. EdgeConv factorized as h[p,i,s] = relu(U[p,i] + V[p, idx[p,i,s]] + ba),
out = max_s (Wb h + bb).

Neighbor access is done fully on-chip: each problem's neighbors come from its
own 30-node window, so V[p, idx] = Vrows_p @ S_p with S_p a [30, 450] one-hot
selection matrix. S is built by DMA-broadcasting the idx row of each problem
across 30 partitions and comparing against an iota constant on VectorE
(is_equal, 4x bf16 mode). The expansion + U-broadcast ride PE PSUM
accumulation; ScalarE drains with fused ReLU+bias; PE applies Wb; VectorE
max-reduces over the 15 slots. No HBM gather, no index round-trip.
"""
import sys

sys.path.insert(0, "/opt/trn_rl_repo")

import numpy as np
import ml_dtypes

import concourse.bass as bass
from concourse import bacc
import concourse.mybir as mybir
import concourse.tile as tile
from concourse.bass_utils import run_bass_kernel_spmd
from concourse.vector_clock import ScopedClock

f32 = mybir.dt.float32
f32r = mybir.dt.float32r
bf16 = mybir.dt.bfloat16
u16 = mybir.dt.uint16
AF = mybir.ActivationFunctionType
OP = mybir.AluOpType
AX = mybir.AxisListType

BS, N, K, HID, EMB = 512, 30, 15, 128, 64
CORES = 8
BSC = BS // CORES          # samples per core
P = 2 * BSC                # 128 problems per core (2 Q-nets)
COLS = P * N               # 3840


def _patched_drain_and_barrier(self, tick_clock, wait_clock):
    # this walrus build caps sync-waits at 1/instruction; spread the
    # end-of-kernel waits over SP NOPs instead of one multi-wait Drain.
    nc = self.nc
    probe = nc.sync.nop()
    wait_clock.add_sem_waits(probe.ins, ScopedClock({None: tick_clock.global_clock}))
    si = probe.ins.sync_info
    waits = list(si.on_wait) if si is not None else []
    if len(waits) > 1:
        si.on_wait = [waits[0]]
        for w in waits[1:]:
            extra = nc.sync.nop()
            esi = extra.ins.sync_info
            if esi is None:
                extra.ins.sync_info = mybir.SyncInfo(on_wait=[w], on_update=[])
            else:
                esi.on_wait = [w]
    nc.sync.drain()
    nc.all_engine_barrier()
    assert self.sems is not None
    popped = nc._tile_sem_poison_stack.pop()
    assert popped is self._sem_poison
    nc.clear_and_free_semaphores(list(self.sems.allocated().values()))
    nc.all_engine_barrier()


tile.TileContext._drain_and_barrier = _patched_drain_and_barrier

_ws_cnt = [0]


def split_excess_waits(nc):
    """This walrus build supports at most 1 sync-wait per instruction (2 for
    EventSemaphore). Tile attaches several; move the extras onto same-engine
    NOPs inserted right before the instruction."""
    for fn in nc.m.functions:
        for bb in fn.blocks:
            new_list = []
            for inst in bb.instructions:
                si = inst.sync_info
                cap = 2 if isinstance(inst, mybir.InstEventSemaphore) else 1
                if si is not None and si.on_wait is not None and len(si.on_wait) > cap:
                    waits = list(si.on_wait)
                    for w in waits[:-cap]:
                        n = mybir.InstNoOp(name=f"I-wsplit-{_ws_cnt[0]}", ins=[], outs=[])
                        _ws_cnt[0] += 1
                        n.engine = inst.engine
                        n.sync_info = mybir.SyncInfo(on_wait=[w], on_update=[])
                        nc.register_instruction(n, overwrite=True)
                        new_list.append(n)
                    si.on_wait = waits[-cap:]
                new_list.append(inst)
            bb.instructions[:] = new_list


def ap_of(t, offset, dims):
    """Build a raw AP on tile/dram tensor t: dims = [[step, count], ...] (elements)."""
    base = t.ap() if hasattr(t, "ap") and not isinstance(t, bass.AP) else t
    return bass.AP(tensor=base.tensor, offset=base.offset + offset, ap=dims)


def build_program():
    nc = bacc.Bacc("TRN2", debug=False)

    din = {}
    def inp(name, shape, dtype=f32):
        din[name] = nc.dram_tensor(name, shape, dtype, kind="ExternalInput")
        return din[name]

    inp("x_T", [4, COLS])
    inp("x_pp", [P, N * 4])
    inp("wm1aT", [2 * (HID + EMB + 2), HID])   # (388, 128)
    inp("wm1bT", [HID, HID])
    inp("wi1T", [4, HID]); inp("wi2T", [4, HID])
    inp("emb1T", [EMB, 3]); inp("emb2T", [EMB, 3])
    inp("wca1T", [HID, HID]); inp("wca2T", [HID, HID])
    inp("wcb1T", [HID, 1]); inp("wcb2T", [HID, 1])
    inp("bi1", [HID, 1]); inp("bi2", [HID, 1])
    inp("bm1a", [HID, 1]); inp("bm1b", [HID, 1])
    inp("bca1", [HID, 1]); inp("bca2", [HID, 1])
    inp("bcb1", [1, 1]); inp("bcb2", [1, 1])
    inp("iden", [HID, HID])
    inp("iota120", [120, 1], f32)

    qout = nc.dram_tensor("qout", [1, COLS], f32, kind="ExternalOutput")

    from contextlib import ExitStack
    ctx = ExitStack()
    with tile.TileContext(nc) as tc, ctx:
        consts = ctx.enter_context(tc.tile_pool(name="consts", bufs=1))
        big = ctx.enter_context(tc.tile_pool(name="big", bufs=1))
        knn = ctx.enter_context(tc.tile_pool(name="knn", bufs=1))
        t8p = ctx.enter_context(tc.tile_pool(name="t8p", bufs=8))
        vrp = ctx.enter_context(tc.tile_pool(name="vrp", bufs=1))
        bcp = ctx.enter_context(tc.tile_pool(name="bcp", bufs=3))
        sp = ctx.enter_context(tc.tile_pool(name="sp", bufs=3))
        hp = ctx.enter_context(tc.tile_pool(name="hp", bufs=3))
        psp = ctx.enter_context(tc.tile_pool(name="psp", bufs=4, space="PSUM"))

        _psn = [0]

        def ps_tile():
            _psn[0] += 1
            return psp.tile([HID, 1024], f32, tag="ps", name=f"ps_{_psn[0]}")

        # ---- load inputs to SBUF ----
        sb = {}
        for name, parts, width, dt in (
            ("x_T", 4, COLS, f32r), ("x_pp", P, N * 4, f32),
            ("wm1bT", HID, HID, f32r),
            ("wi1T", 4, HID, f32r), ("wi2T", 4, HID, f32r),
            ("emb1T", EMB, 3, f32), ("emb2T", EMB, 3, f32),
            ("wca1T", HID, HID, f32r), ("wca2T", HID, HID, f32r),
            ("wcb1T", HID, 1, f32r), ("wcb2T", HID, 1, f32r),
            ("bi1", HID, 1, f32), ("bi2", HID, 1, f32),
            ("bm1a", HID, 1, f32), ("bm1b", HID, 1, f32),
            ("bca1", HID, 1, f32), ("bca2", HID, 1, f32),
            ("bcb1", 1, 1, f32), ("bcb2", 1, 1, f32),
            ("iden", HID, HID, f32r),
            ("iota120", 120, 1, f32),
        ):
            t = consts.tile([parts, width], dt, tag=name)
            src = din[name].ap()
            if dt == f32r:
                src = src.bitcast(f32r)
            nc.sync.dma_start(out=t[:], in_=src)
            sb[name] = t
        # wm1aT slices: a1a/a2a = xi/xj-xi parts for the 128 init feats,
        # a1b/a2b = same for the 64 class-embedding feats (tar rows are zero
        # features -> dropped entirely).
        wa_sl = {}
        for nm, lo, hi in (("a1a", 0, 128), ("a1b", 128, 192),
                           ("a2a", 194, 322), ("a2b", 322, 386)):
            t = consts.tile([hi - lo, HID], f32r, tag="wa_" + nm)
            nc.sync.dma_start(out=t[:], in_=din["wm1aT"].ap()[lo:hi, :].bitcast(f32r))
            wa_sl[nm] = t
        wau1 = consts.tile([128, HID], f32r)
        wau2 = consts.tile([64, HID], f32r)
        nc.gpsimd.tensor_tensor(out=wau1[:], in0=wa_sl["a1a"][:],
                                in1=wa_sl["a2a"][:], op=OP.subtract)
        nc.gpsimd.tensor_tensor(out=wau2[:], in0=wa_sl["a1b"][:],
                                in1=wa_sl["a2b"][:], op=OP.subtract)

        # ---- phase A: init features (feature-on-partition) ----
        initT1 = big.tile([HID, COLS], f32r, tag="bigA")
        initT2 = big.tile([64, COLS], f32r)
        clsr = {}
        for q in (0, 1):
            t = consts.tile([EMB, 3], f32, tag=f"clsr{q}")
            nc.scalar.activation(t[:], sb["emb1T" if q == 0 else "emb2T"][:], AF.Relu)
            clsr[q] = t
        for q in (0, 1):
            # initT2 <- relu(cls) columns by category (i//10), bcast over prob
            dst = bass.AP(tensor=initT2.tensor, offset=initT2.offset + q * (COLS // 2),
                          ap=[[initT2.ap[0][0], 64], [N, BSC], [10, 3], [1, 10]])
            src = bass.AP(tensor=clsr[q].tensor, offset=clsr[q].offset,
                          ap=[[clsr[q].ap[0][0], 64], [0, BSC], [1, 3], [0, 10]])
            nc.gpsimd.tensor_copy(dst, src)
        for c in range(8):
            q = 0 if c < 4 else 1
            ps = ps_tile()
            nc.tensor.matmul(ps[:, :480], lhsT=sb["wi1T" if q == 0 else "wi2T"][:],
                             rhs=sb["x_T"][:, c * 480:(c + 1) * 480],
                             start=True, stop=True)
            nc.scalar.activation(initT1[:, c * 480:(c + 1) * 480], ps[:, :480], AF.Relu,
                                 bias=sb["bi1" if q == 0 else "bi2"][:])

        # ---- phase B: U^T (feature-major) ----
        UT = big.tile([HID, COLS], f32r)
        for c in range(8):
            ps = ps_tile()
            sl = slice(c * 480, (c + 1) * 480)
            nc.tensor.matmul(ps[:, :480], lhsT=wau1[:],
                             rhs=initT1[:, sl], start=True, stop=False)
            nc.tensor.matmul(ps[:, :480], lhsT=wau2[:],
                             rhs=initT2[:, sl], start=False, stop=True)
            nc.scalar.activation(UT[:, sl], ps[:, :480], AF.Copy)

        # ---- phase B2: V^T wide matmuls, then PE-transpose to V rows ----
        VT = big.tile([HID, COLS], f32r)
        for c in range(8):
            ps = ps_tile()
            sl = slice(c * 480, (c + 1) * 480)
            nc.tensor.matmul(ps[:, :480], lhsT=wa_sl["a2a"][:],
                             rhs=initT1[:, sl], start=True, stop=False)
            nc.tensor.matmul(ps[:, :480], lhsT=wa_sl["a2b"][:],
                             rhs=initT2[:, sl], start=False, stop=True)
            nc.scalar.activation(VT[:, sl], ps[:, :480], AF.Copy)
        vrows = []
        for g in range(32):
            ps = ps_tile()
            nc.tensor.matmul(ps[0:120, 0:HID].bitcast(f32r),
                             lhsT=VT[:, g * 120:(g + 1) * 120],
                             rhs=sb["iden"][:], is_transpose=True,
                             start=True, stop=True)
            vr = vrp.tile([120, HID], bf16, tag=f"vr{g}")
            nc.scalar.activation(vr[:], ps[0:120, 0:HID].bitcast(f32r), AF.Copy)
            vrows.append(vr)

        # ---- phase C: kNN + top-15 ----
        diff = knn.tile([P, 3600], f32)
        xpp = sb["x_pp"]
        nc.vector.tensor_sub(
            diff[:],
            ap_of(xpp, 0, [list(xpp.ap[0]), [4, N], [0, N], [1, 4]]),
            ap_of(xpp, 0, [list(xpp.ap[0]), [0, N], [4, N], [1, 4]]))
        negsq = knn.tile([P, 3600], f32)
        nc.scalar.square(negsq[:], diff[:])
        negd = knn.tile([P, 900], f32)
        nc.vector.tensor_reduce(
            out=negd[:], in_=ap_of(negsq, 0, [list(negsq.ap[0]), [4, 900], [1, 4]]),
            axis=AX.X, op=OP.add, negate=True)
        nc.vector.memset(ap_of(negd, 0, [list(negd.ap[0]), [31, N]]), -1e30)

        idxall = knn.tile([P, 480], u16)
        for i in range(N):
            nd = negd[:, i * 30:(i + 1) * 30]
            m8 = t8p.tile([P, 8], f32, tag="m8")
            scr = t8p.tile([P, 30], f32, tag="scr")
            m8b = t8p.tile([P, 8], f32, tag="m8b")
            nc.vector.max(m8[:], nd)
            nc.vector.max_index(idxall[:, i * 16:i * 16 + 8], m8[:], nd)
            nc.vector.match_replace(scr[:], in_to_replace=m8[:], in_values=nd,
                                    imm_value=-1e30)
            nc.vector.max(m8b[:], scr[:])
            nc.vector.max_index(idxall[:, i * 16 + 8:i * 16 + 16], m8b[:], scr[:])

        # gfp: per-problem local neighbor idx, col = s*30 + i (bf16, exact)
        gfp = knn.tile([P, N * K], bf16)
        nc.vector.tensor_copy(
            ap_of(gfp, 0, [list(gfp.ap[0]), [30, 8], [1, N]]),
            ap_of(idxall, 0, [list(idxall.ap[0]), [1, 8], [16, N]]))
        nc.vector.tensor_copy(
            ap_of(gfp, 8 * 30, [list(gfp.ap[0]), [30, 7], [1, N]]),
            ap_of(idxall, 8, [list(idxall.ap[0]), [1, 7], [16, N]]))

        # ---- phase D: one-hot expansion + U + mm2 + slot-max, head interleaved
        Hfin = big.tile([HID, COLS], f32)
        Hb = big.tile([HID, COLS], f32r, tag="bigA")  # reuse initT1 slot
        HC = big.tile([HID, COLS], f32r)
        qrow = big.tile([1, COLS], f32)

        def head(c):
            q = 0 if c < 4 else 1
            sl = slice(c * 480, (c + 1) * 480)
            nc.scalar.activation(Hb[:, sl], Hfin[:, sl], AF.Relu, bias=sb["bm1b"][:])
            ps = ps_tile()
            nc.tensor.matmul(ps[:, :480], lhsT=sb["wca1T" if q == 0 else "wca2T"][:],
                             rhs=Hb[:, sl], start=True, stop=True)
            nc.scalar.activation(HC[:, sl], ps[:, :480], AF.Relu,
                                 bias=sb["bca1" if q == 0 else "bca2"][:])
            ps2 = ps_tile()
            nc.tensor.matmul(ps2[0:1, 0:480], lhsT=sb["wcb1T" if q == 0 else "wcb2T"][:],
                             rhs=HC[:, sl], start=True, stop=True)
            nc.scalar.activation(qrow[:, sl], ps2[0:1, 0:480], AF.Identity,
                                 bias=sb["bcb1" if q == 0 else "bcb2"][:])

        # broadcast staging tiles: off-band cells stay 99 (never matches iota)
        # so S4 columns are one-hot within their own problem's 30-row band and
        # zero elsewhere -> one [120,128] lhsT serves all 4 problems.
        bcs = []
        for r in range(3):
            t = big.tile([120, 4 * 450], bf16, tag=f"bc{r}")
            nc.gpsimd.memset(t[:], 99.0)
            bcs.append(t)

        for g in range(32):
            # emit block heads 2 groups late so their deps (the block's last
            # slot-max) are long done and they never stall the engine queues
            if g >= 6 and (g - 6) % 4 == 0:
                head((g - 6) // 4)
            bc4 = bcs[g % 3]
            for qq in range(4):
                src = ap_of(gfp, (4 * g + qq) * gfp.ap[0][0],
                            [[gfp.ap[0][0], 1], [0, 30], [1, 450]])
                nc.sync.dma_start(
                    out=bc4[qq * 30:(qq + 1) * 30, qq * 450:(qq + 1) * 450],
                    in_=src)
            S4 = sp.tile([120, 4 * 450], bf16, tag="S4")
            eng = nc.vector if g % 2 == 0 else nc.gpsimd
            eng.tensor_scalar(out=S4[:], in0=bc4[:], scalar1=sb["iota120"][:],
                              scalar2=None, op0=OP.is_equal)
            for pair in range(2):
                psx = ps_tile()
                for qq in (2 * pair, 2 * pair + 1):
                    qg = 4 * g + qq
                    co = 512 * (qq - 2 * pair)
                    nc.tensor.matmul(psx[:, co:co + 450],
                                     lhsT=vrows[g][:],
                                     rhs=S4[:, qq * 450:(qq + 1) * 450],
                                     start=True, stop=False)
                    uap = ap_of(UT, qg * 30,
                                [list(UT.ap[0]), [0, 15], [1, 30]]).bitcast(f32r)
                    nc.tensor.matmul(psx[:, co:co + 450], lhsT=sb["iden"][:],
                                     rhs=uap, start=False, stop=True)
                h2 = hp.tile([HID, 900], f32r, tag="h2")
                nc.scalar.activation(
                    h2[:], ap_of(psx, 0, [list(psx.ap[0]), [512, 2], [1, 450]]),
                    AF.Relu, bias=sb["bm1a"][:])
                # mm2 reuses the expansion PSUM tile (relu already drained it)
                # to halve PSUM pressure -> deeper cross-pair pipelining.
                for k in range(2):
                    nc.tensor.matmul(psx[:, 512 * k:512 * k + 450],
                                     lhsT=sb["wm1bT"][:],
                                     rhs=h2[:, 450 * k:450 * k + 450],
                                     start=True, stop=True)
                qg0 = 4 * g + 2 * pair
                nc.vector.tensor_reduce(
                    out=Hfin[:, qg0 * 30:qg0 * 30 + 60],
                    in_=ap_of(psx, 0, [list(psx.ap[0]), [512, 2], [1, 30], [30, 15]]),
                    axis=AX.X, op=OP.max)
        head(6)
        head(7)

        nc.sync.dma_start(out=qout.ap(), in_=qrow[:])

    nc.compile()
    split_excess_waits(nc)
    return nc


_CACHED = {}


def _get_program():
    if "nc" not in _CACHED:
        _CACHED["nc"] = build_program()
    return _CACHED["nc"]


def _host_inputs(state, action, weights):
    nodes1 = np.concatenate(
        [state.reshape(BS, N, 2), action.reshape(BS, N, 2)], axis=-1)
    nodes2 = np.concatenate([state, action], axis=1).reshape(BS, N, 4)
    iden = np.eye(HID, dtype=np.float32)
    shared = {
        "wm1aT": np.ascontiguousarray(weights["W_m1a"].T),
        "wm1bT": np.ascontiguousarray(weights["W_m1b"].T),
        "wi1T": np.ascontiguousarray(weights["W_init1"].T),
        "wi2T": np.ascontiguousarray(weights["W_init2"].T),
        "emb1T": np.ascontiguousarray(weights["emb1"].T),
        "emb2T": np.ascontiguousarray(weights["emb2"].T),
        "wca1T": np.ascontiguousarray(weights["W_c1a"].T),
        "wca2T": np.ascontiguousarray(weights["W_c2a"].T),
        "wcb1T": np.ascontiguousarray(weights["W_c1b"].T),
        "wcb2T": np.ascontiguousarray(weights["W_c2b"].T),
        "bi1": weights["b_init1"].reshape(HID, 1),
        "bi2": weights["b_init2"].reshape(HID, 1),
        "bm1a": weights["b_m1a"].reshape(HID, 1),
        "bm1b": weights["b_m1b"].reshape(HID, 1),
        "bca1": weights["b_c1a"].reshape(HID, 1),
        "bca2": weights["b_c2a"].reshape(HID, 1),
        "bcb1": weights["b_c1b"].reshape(1, 1),
        "bcb2": weights["b_c2b"].reshape(1, 1),
        "iden": iden,
        "iota120": (np.arange(120, dtype=np.float32) % 30).reshape(120, 1),
    }
    shared = {k: np.ascontiguousarray(v, dtype=v.dtype) for k, v in shared.items()}
    in_maps = []
    for c in range(CORES):
        x_pp = np.concatenate(
            [nodes1[c * BSC:(c + 1) * BSC], nodes2[c * BSC:(c + 1) * BSC]], axis=0)
        x_T = np.ascontiguousarray(x_pp.transpose(2, 0, 1).reshape(4, COLS))
        m = dict(shared)
        m["x_pp"] = np.ascontiguousarray(x_pp.reshape(P, N * 4))
        m["x_T"] = x_T
        in_maps.append(m)
    return in_maps


def kernel(**inputs):
    state = np.asarray(inputs["state"], np.float32)
    action = np.asarray(inputs["action"], np.float32)
    weights = {k: np.asarray(v, np.float32) for k, v in inputs.items()
               if k not in ("state", "action")}
    nc = _get_program()
    in_maps = _host_inputs(state, action, weights)
    res = run_bass_kernel_spmd(nc, in_maps, core_ids=list(range(CORES)))
    q1 = np.zeros((BS, N), np.float32)
    q2 = np.zeros((BS, N), np.float32)
    for c in range(CORES):
        probs = res.results[c]["qout"].reshape(P, N)   # col = prob*30 + node
        q1[c * BSC:(c + 1) * BSC] = probs[:BSC]
        q2[c * BSC:(c + 1) * BSC] = probs[BSC:]
    return (q1, q2)


if __name__ == "__main__":
    print("smoke build only")
    build_program()
    print("built ok")
